# revision 8
# baseline (speedup 1.0000x reference)
"""Trainium2 Bass kernel for MinibatchDiscrimination.

Reference computation (B=256, IN=1024, O=64, K=50):
    M = (x @ T).reshape(B, O, K)
    l1[i,j,o] = sum_k |M[i,o,k] - M[j,o,k]|
    out = concat([x, sum_j exp(-l1) - 1], axis=1)          # [B, IN + O]

Sharding: the O (out_features) dimension is split across the 8 NeuronCores
(8 features per core); x is replicated. Each core computes its [256, 8]
feature block; the host gathers the blocks and concatenates with x.

Per-core pipeline (v2 — fp8 DoubleRow everywhere on the PE):
  1. PE DoubleRow GEMM: M[256, 400] = xT.T @ T_local (fp8 in, f32 PSUM,
     contraction 1024 = 4 passes x 2 planes x 128), cast to fp8 — the
     canonical value used on BOTH sides of the pairwise subtraction, so the
     diagonal distance is exactly zero. -M is staged to DRAM.
  2. All-pairs signed differences generated by the PE with an affine
     DoubleRow matmul: contraction 52 = 2 planes x 26 partitions
     (rows 0..49 = I50 k-rows, row 50 = affine ones/-M row, row 51 zero).
     0.5 cycles per output column. Chunks of 32 j land in PSUM as
     [128, 4x512] f32 banks (400 of each 512-col bank used: 8 j x 50 k).
  3. D[i,j] = D[j,i] symmetry: itile-1 computes only j in [128,256); the
     mirrored contribution comes from PE column-sums of the itile-0 exp
     tiles at the end.
  4. Each PSUM chunk takes one of three abs+k-reduce paths (balancing DVE,
     ScalarE and GPSIMD): (A) DVE tensor_reduce(add, abs) straight from
     PSUM; (B) ScalarE Abs-cast to bf16 SBUF + DVE halve-add (2x) +
     reduce; (C) ScalarE Abs-cast + GPSIMD binary-tree adds.
  5. ScalarE exp(-l1) (scale=-1) with fused accum_out giving the j-sum
     per feature directly; -1.0, DMA out.
"""

import numpy as np
import ml_dtypes

B = 256
IN_FEATURES = 1024
O_TOTAL = 64
K = 50
N_CORES = 8
O_LOC = O_TOTAL // N_CORES          # 8 features per core
N_LOC = O_LOC * K                   # 400 M columns per core
P = 128                             # partitions
ITILES = B // P                     # 2 row tiles
CC = IN_FEATURES // P               # 8 contraction chunks
KP = 26                             # DoubleRow partitions (2 planes of 26)
KB = K * B                          # 12800 diff columns per plane
JCHUNK = 32                         # j's per PSUM chunk
JB = 8                              # j's per PSUM bank (8*50 = 400 of 512)
QB = JCHUNK // JB                   # banks per chunk = 4
NCHUNK = B // JCHUNK                # 8 chunks per full block
CH = QB * 512                       # 2048 PSUM elements per chunk
PATTERN = "ABABC"                   # elem-path cycle: DVE / Sc+DVE / Sc+GP

_cache = {}


def _build_program():
    import concourse.mybir as mybir
    from concourse import bacc, tile
    from concourse.masks import make_identity

    f32 = mybir.dt.float32
    bf16 = mybir.dt.bfloat16
    fp8 = mybir.dt.float8e4
    Alu = mybir.AluOpType
    Act = mybir.ActivationFunctionType
    DR = mybir.MatmulPerfMode.DoubleRow

    nc = bacc.Bacc("TRN2", target_bir_lowering=False, debug=False,
                   enable_asserts=False)

    xT_d = nc.dram_tensor("xT", [IN_FEATURES, B], fp8, kind="ExternalInput").ap()
    T_d = nc.dram_tensor("Tl", [IN_FEATURES, N_LOC], fp8, kind="ExternalInput").ap()
    rp_d = nc.dram_tensor("rp", [KP + 2, 2 * KB], fp8,
                          kind="ExternalInput").ap()
    feat_d = nc.dram_tensor("feat", [B, O_LOC], f32, kind="ExternalOutput").ap()

    with tile.TileContext(nc) as tc:
        with (
            tc.tile_pool(name="static", bufs=1) as static,
            tc.tile_pool(name="bap", bufs=4) as bap,
            tc.tile_pool(name="hp", bufs=3) as hp,
            tc.tile_pool(name="scp", bufs=2) as scp,
            tc.tile_pool(name="dexpp", bufs=2) as dexpp,
            tc.tile_pool(name="et0p", bufs=8) as et0p,
            tc.tile_pool(name="et1p", bufs=2) as et1p,
            tc.tile_pool(name="dramp", bufs=1, space="DRAM") as dramp,
        ):
            # ---- rhs identity planes load first: they gate the pairwise ----
            rhs_t = []
            for h in range(2):
                rt = static.tile([KP, 2 * KB], fp8, tag=f"rhs{h}",
                                 name=f"rhs{h}")
                nc.sync.dma_start(out=rt[:, 0:KB],
                                  in_=rp_d[0:KP, 0:KB])
                nc.gpsimd.dma_start(out=rt[:, KB:],
                                    in_=rp_d[0:KP, KB:])
                rhs_t.append(rt)

            # ---- stage 1: load inputs, M = x @ T_local (DoubleRow GEMM) ----
            xt_sb = static.tile([P, CC * B], fp8, tag="xt")
            t_sb = static.tile([P, CC * N_LOC], fp8, tag="t")
            for cc in range(CC):
                nc.sync.dma_start(out=xt_sb[:, cc * B:(cc + 1) * B],
                                  in_=xT_d[cc * P:(cc + 1) * P, :])
                nc.gpsimd.dma_start(out=t_sb[:, cc * N_LOC:(cc + 1) * N_LOC],
                                    in_=T_d[cc * P:(cc + 1) * P, :])

            warm = static.tile([1, 2], f32, tag="warm")
            nc.vector.memset(warm[:, :], 0.0)
            nc.scalar.activation(out=warm[:, :], in_=warm[:, :],
                                 func=Act.Exp, scale=-1.0)
            ident = static.tile([P, P], fp8, tag="ident")
            make_identity(nc, ident[:, :])
            identf = static.tile([JB, JB], f32, tag="identf")
            make_identity(nc, identf[:, :])
            ones_col = static.tile([P, 1], f32, tag="ones_col")
            nc.vector.memset(ones_col[:, :], 1.0)

            # -M staged to DRAM as one flat j-major row per o, so the
            # per-o rhs row refresh is a single contiguous 12.8KB packet
            negm_d = dramp.tile([O_LOC, KB], fp8, tag="negm_d")
            m_bf = []
            ngs = []
            with tc.tile_pool(name="mmp", bufs=2, space="PSUM") as mmp:
                for it in range(ITILES):
                    pm = mmp.tile([P, N_LOC], f32, tag="pm")
                    for g in range(CC // 2):
                        lhsT = xt_sb[:, g * 2 * B: (g + 1) * 2 * B].rearrange(
                            "p (two i) -> p two i", two=2)[
                            :, :, it * P:(it + 1) * P]
                        rhs = t_sb[:, g * 2 * N_LOC:(g + 1) * 2 * N_LOC].\
                            rearrange("p (two n) -> p two n", two=2)
                        nc.tensor.matmul(
                            pm[:, :], lhsT=lhsT, rhs=rhs,
                            start=(g == 0), stop=(g == CC // 2 - 1),
                            perf_mode=DR,
                        )
                    mb = static.tile([P, N_LOC], fp8, tag=f"mbf{it}",
                                     name=f"mbf{it}")
                    nc.scalar.copy(mb[:, :], pm[:, :])
                    m_bf.append(mb)
                    ng = static.tile([P, N_LOC], fp8, tag=f"neg{it}",
                                     name=f"neg{it}")
                    nc.vector.tensor_scalar(out=ng[:, :], in0=mb[:, :],
                                            scalar1=-1.0, scalar2=None,
                                            op0=Alu.mult)
                    ngs.append(ng)
                half = K * P
                for o in range(O_LOC):
                    for it in range(ITILES):
                        nc.sync.dma_start(
                            out=negm_d[o:o + 1,
                                       it * half:(it + 1) * half],
                            in_=ngs[it][:, o * K:(o + 1) * K])

            # ---- stage 2: lhsT tiles [KP, 2 planes x 256] ------------------
            # plane0 = M_o^T rows 0..25; plane1 rows 0..23 = M_o^T rows
            # 26..49, row 24 = ones (affine), row 25 = 0
            # fp8 PE transposes in two k-slices (26 + 24 rows) so each lands
            # at base partition 0; fp8 transpose output needs element step 2
            lhs = []
            with tc.tile_pool(name="tpp", bufs=4, space="PSUM") as tpp:
                for o in range(O_LOC):
                    lt = static.tile([KP, 2 * B], fp8, tag=f"lhs{o}",
                                     name=f"lhs{o}")
                    for it in range(ITILES):
                        for pl, (k0, k1) in enumerate([(0, KP), (KP, K)]):
                            nk = k1 - k0
                            tp = tpp.tile([KP, 2 * P], fp8, tag="tp")
                            tpv = tp[0:nk, :].rearrange(
                                "p (i two) -> p i two", two=2)
                            nc.tensor.transpose(
                                tpv[:, :, 0:1],
                                m_bf[it][:, o * K + k0: o * K + k1],
                                ident[:, :])
                            dst = lt[0:nk, pl * B + it * P:
                                     pl * B + (it + 1) * P].rearrange(
                                "p (i one) -> p i one", one=1)
                            if (it + pl) % 2 == 0:
                                nc.scalar.copy(dst, tpv[:, :, 0:1])
                            else:
                                nc.vector.tensor_copy(out=dst,
                                                      in_=tpv[:, :, 0:1])
                    nc.sync.dma_start(out=lt[24:25, B:2 * B],
                                      in_=rp_d[KP:KP + 1, 0:B])
                    nc.sync.dma_start(out=lt[25:26, B:2 * B],
                                      in_=rp_d[KP + 1:KP + 2, 0:B])
                    lhs.append(lt)

            # ---- stage 4: per (o, itile): diffs -> |.| -> k-sum -> exp -----
            feat_sb = [static.tile([P, O_LOC], f32, tag=f"feat{it}",
                                   name=f"feat{it}")
                       for it in range(ITILES)]
            et0_tiles = []
            gidx = 0
            stage4 = tc.tile_pool(name="chp", bufs=2, space="PSUM")
            chp = stage4.__enter__()
            for o in range(O_LOC):
                rt = rhs_t[o % 2]
                nc.sync.dma_start(out=rt[24:25, KB:2 * KB],
                                  in_=negm_d[o:o + 1, :])
                rtv = rt[:, :].rearrange("p (two n) -> p two n", two=2)
                ltv = lhs[o][:, :].rearrange("p (two i) -> p two i", two=2)
                for it in range(ITILES):
                    c_lo = 0 if it == 0 else NCHUNK // 2
                    nj = (NCHUNK - c_lo) * JCHUNK
                    dexp = dexpp.tile([P, B], f32, tag="dexp")
                    for c in range(c_lo, NCHUNK):
                        ch = chp.tile([P, CH], f32, tag="ch")
                        for q in range(QB):
                            col = (c * JCHUNK + q * JB) * K
                            nc.tensor.matmul(
                                ch[:, q * 512: q * 512 + JB * K],
                                lhsT=ltv[:, :, it * P:(it + 1) * P],
                                rhs=rtv[:, :, col: col + JB * K],
                                start=True, stop=True, perf_mode=DR)
                        # PSUM chunk viewed [p, q(4), j(8), k(50)]
                        ch4 = ch[:, :].rearrange(
                            "p (q r) -> p q r", q=QB)[
                            :, :, 0:JB * K].rearrange(
                            "p q (j k) -> p q j k", k=K)
                        gsl = dexp[:, (c - c_lo) * JCHUNK:
                                   (c - c_lo + 1) * JCHUNK]
                        path = PATTERN[gidx % len(PATTERN)]
                        gidx += 1
                        if path == "A":
                            # DVE: fused |.| + k-reduce from PSUM
                            nc.vector.tensor_reduce(
                                out=gsl.rearrange("p (q j) -> p q j", q=QB),
                                in_=ch4,
                                axis=mybir.AxisListType.X, op=Alu.add,
                                apply_absolute_value=True)
                            continue
                        # ScalarE |.| cast to bf16 (dense j-major)
                        ba = bap.tile([P, JCHUNK * K], bf16, tag="ba")
                        ba3 = ba[:, :].rearrange("p (j k) -> p j k", k=K)
                        nc.scalar.activation(
                            out=ba3.rearrange("p (q j) k -> p q j k", q=QB),
                            in_=ch4, func=Act.Abs)
                        if path == "B":
                            # DVE: one 2x halve-add + reduce over 25
                            h = hp.tile([P, JCHUNK * 25], bf16, tag="h")
                            h3 = h[:, :].rearrange("p (j k) -> p j k", k=25)
                            nc.vector.tensor_tensor(
                                out=h3, in0=ba3[:, :, 0:25],
                                in1=ba3[:, :, 25:50], op=Alu.add)
                            nc.vector.tensor_reduce(
                                out=gsl, in_=h3,
                                axis=mybir.AxisListType.X, op=Alu.add)
                        else:
                            # GPSIMD binary-tree adds (post-abs, SBUF only)
                            sc = scp.tile([P, 1664], bf16, tag="sc")
                            def lv(ofs, w):
                                return sc[:, ofs: ofs + JCHUNK * w].rearrange(
                                    "p (j k) -> p j k", k=w)
                            L25, L12, L6, L3, L1, T1 = (
                                lv(0, 25), lv(800, 12), lv(1184, 6),
                                lv(1376, 3), lv(1472, 1), lv(1504, 1))
                            gp = nc.gpsimd
                            gp.tensor_tensor(out=L25, in0=ba3[:, :, 0:25],
                                             in1=ba3[:, :, 25:50], op=Alu.add)
                            gp.tensor_tensor(out=L12, in0=L25[:, :, 0:12],
                                             in1=L25[:, :, 12:24], op=Alu.add)
                            gp.tensor_tensor(out=L6, in0=L12[:, :, 0:6],
                                             in1=L12[:, :, 6:12], op=Alu.add)
                            gp.tensor_tensor(out=L3, in0=L6[:, :, 0:3],
                                             in1=L6[:, :, 3:6], op=Alu.add)
                            gp.tensor_tensor(out=L1, in0=L3[:, :, 0:1],
                                             in1=L3[:, :, 1:2], op=Alu.add)
                            gp.tensor_tensor(out=T1, in0=L1,
                                             in1=L3[:, :, 2:3], op=Alu.add)
                            gp.tensor_tensor(
                                out=gsl.rearrange("p (j k) -> p j k", k=1),
                                in0=T1, in1=L25[:, :, 24:25], op=Alu.add)
                    if it == 0:
                        et = et0p.tile([P, B], f32, tag="et0",
                                       name=f"et0_{o}")
                        et0_tiles.append(et)
                        nc.scalar.activation(
                            out=et[:, :], in_=dexp[:, 0:nj],
                            func=Act.Exp, scale=-1.0,
                            accum_out=feat_sb[0][:, o:o + 1])
                    else:
                        et = et1p.tile([P, B // 2], f32, tag="et1")
                        nc.scalar.activation(
                            out=et[:, :], in_=dexp[:, 0:nj],
                            func=Act.Exp, scale=-1.0,
                            accum_out=feat_sb[1][:, o:o + 1])
            stage4.__exit__(None, None, None)

            # ---- stage 5: mirrored contribution for itile 1 ----------------
            # colsum_o[j] = sum_{i in it0} exp(-D[i, j]) for j in [128, 256)
            cs_sb = static.tile([JB, P], f32, tag="cs_sb")
            with tc.tile_pool(name="csp", bufs=2, space="PSUM") as csp:
                for o in range(O_LOC):
                    cs = csp.tile([1, P], f32, tag="cs")
                    nc.tensor.matmul(cs[:, :], lhsT=ones_col[:, :],
                                     rhs=et0_tiles[o][:, P:B],
                                     start=True, stop=True)
                    cs_row = hp.tile([1, P], f32, tag="cs_row")
                    nc.scalar.copy(cs_row[:, :], cs[:, :])
                    nc.sync.dma_start(out=cs_sb[o:o + 1, :], in_=cs_row[:, :])
                ct = csp.tile([P, JB], f32, tag="ct")
                nc.tensor.transpose(ct[:, :], cs_sb[:, :], identf[:, :])
                nc.vector.tensor_tensor(out=feat_sb[1][:, :],
                                        in0=feat_sb[1][:, :],
                                        in1=ct[:, :], op=Alu.add)

            for it in range(ITILES):
                nc.vector.tensor_scalar(
                    out=feat_sb[it][:, :], in0=feat_sb[it][:, :],
                    scalar1=1.0, scalar2=None, op0=Alu.subtract)
                nc.sync.dma_start(out=feat_d[it * P:(it + 1) * P, :],
                                  in_=feat_sb[it][:, :])

    nc.compile()
    return nc


def _get_program():
    if "nc" not in _cache:
        _cache["nc"] = _build_program()
    return _cache["nc"]


def prepare_in_maps(x, T):
    """Host-side sharding: transpose/cast x, slice T per core, build the
    DoubleRow identity-plane rhs pattern."""
    f8 = ml_dtypes.float8_e4m3fn
    xT = np.ascontiguousarray(np.asarray(x, dtype=np.float32).T).astype(f8)
    Tf = np.asarray(T, dtype=np.float32)
    in_maps = []
    rp = np.zeros((KP + 2, 2 * KB), dtype=f8)
    jj = np.arange(B)
    for p in range(KP):
        rp[p, jj * K + p] = 1.0            # plane0: k = p (0..25)
    for p in range(K - KP):
        rp[p, KB + jj * K + KP + p] = 1.0  # plane1: k = 26 + p (26..49)
    rp[KP, :] = 1.0                        # ones row for lhs affine
    for c in range(N_CORES):
        Tl = np.ascontiguousarray(
            Tf[:, c * N_LOC:(c + 1) * N_LOC]).astype(f8)
        in_maps.append({"xT": xT, "Tl": Tl, "rp": rp})
    return in_maps


def run_cores(in_maps, trace=False, tmpdir=None):
    from concourse import bass_utils
    nc = _get_program()
    return bass_utils.run_bass_kernel_spmd(
        nc, in_maps, core_ids=list(range(N_CORES)), trace=trace, tmpdir=tmpdir)


def kernel(x, T):
    x = np.asarray(x, dtype=np.float32)
    res = run_cores(prepare_in_maps(x, T))
    feat = np.concatenate(
        [res.results[c]["feat"].astype(np.float32) for c in range(N_CORES)],
        axis=1)
    return np.concatenate([x, feat], axis=1)
